# revision 1
# baseline (speedup 1.0000x reference)
"""Trainium2 Bass kernel for a rate-1/2, constraint-length-3 feedforward
convolutional encoder (generator polynomials "101" and "111", MSB-first).

The trellis scan in the reference collapses to elementwise XORs of shifted
input bits (zero initial state):

    out0[t] = u[t] ^ u[t-2]            (poly "101")
    out1[t] = u[t] ^ u[t-1] ^ u[t-2]   (poly "111")

with the codeword interleaved time-major: y[:, 2t] = out0[t], y[:, 2t+1] = out1[t].

XOR on {0,1} floats is computed arithmetically: x ^ y = (x - y)^2.

Sharding: pure data parallel over the batch dim across 8 NeuronCores.
The kernel is DMA-bound (3 MiB of HBM traffic per 1 MiB of input); the
compute (2 vector + 2 scalar ops per tile) hides entirely under the DMA.
"""

import numpy as np

N_CORES = 8
B, K = 8192, 2048
N_OUT = 2
SHARD_B = B // N_CORES  # 1024 codewords per core
P = 128                 # SBUF partitions

_compiled = {}


def _build_nc():
    import concourse.bass as bass  # noqa: F401
    import concourse.tile as tile
    from concourse import bacc, mybir

    nc = bacc.Bacc(
        "TRN2",
        target_bir_lowering=False,
        debug=False,
        enable_asserts=False,
    )
    x = nc.dram_tensor("x", [SHARD_B, K], mybir.dt.float32, kind="ExternalInput").ap()
    y = nc.dram_tensor(
        "y", [SHARD_B, N_OUT * K], mybir.dt.float32, kind="ExternalOutput"
    ).ap()

    n_groups = SHARD_B // P  # 8 row-groups of 128
    N_SLOTS = 6

    with tile.TileContext(nc) as tc:
        with (
            tc.tile_pool(name="xin", bufs=1) as in_pool,
            tc.tile_pool(name="out", bufs=5) as out_pool,
            tc.tile_pool(name="tmp", bufs=4) as tmp_pool,
        ):
            # Persistent input slots with 2 leading zero columns so the
            # shifted views u[t-1], u[t-2] fall out of plain column offsets.
            # The zero columns are written ONCE here; the per-iteration DMAs
            # only write cols [2:], so no DMA ever waits on a memset.
            in_slots = [
                in_pool.tile(
                    [P, K + 2], mybir.dt.float32, tag=f"xin{j}", name=f"xin{j}"
                )
                for j in range(N_SLOTS)
            ]
            for j in range(N_SLOTS):
                nc.vector.memset(in_slots[j][:, 0:2], 0.0)

            for g in range(n_groups):
                xin = in_slots[g % N_SLOTS]
                rows = slice(g * P, (g + 1) * P)
                # Input DMAs on the SP HWDGE ring (Sync sequencer).
                nc.sync.dma_start(xin[:, 2 : 2 + K], x[rows, :])

                a = xin[:, 2 : 2 + K]  # u[t]
                b = xin[:, 1 : 1 + K]  # u[t-1]
                c = xin[:, 0:K]        # u[t-2]

                out = out_pool.tile(
                    [P, N_OUT * K], mybir.dt.float32, tag="out", name="out"
                )
                even = out[:, 0 : N_OUT * K : 2]
                odd = out[:, 1 : N_OUT * K : 2]

                # p = a - c in {-1,0,1}; out0 = p^2 = a ^ c
                p = tmp_pool.tile([P, K], mybir.dt.float32, tag="p", name="p")
                nc.vector.tensor_tensor(p[:], a, c, mybir.AluOpType.subtract)
                nc.scalar.square(even, p[:])

                # q = out0 - b in {-1,0,1}; out1 = q^2 = out0 ^ b
                # (reuses p's buffer: p is dead once the first square ran)
                nc.vector.tensor_tensor(p[:], even, b, mybir.AluOpType.subtract)
                nc.scalar.square(odd, p[:])

                # Output DMAs on the SWDGE path (GpSimd sequencer) so a
                # stalled input-DMA trigger never blocks a ready output DMA
                # (and vice versa) — the two streams issue independently.
                nc.gpsimd.dma_start(y[rows, :], out[:])

    nc.compile()
    return nc


def _get_nc():
    if "nc" not in _compiled:
        _compiled["nc"] = _build_nc()
    return _compiled["nc"]


def kernel(**inputs) -> np.ndarray:
    from concourse.bass_utils import run_bass_kernel_spmd

    x_full = np.ascontiguousarray(np.asarray(inputs["inputs"], dtype=np.float32))
    assert x_full.shape == (B, K), x_full.shape

    nc = _get_nc()
    in_maps = [
        {"x": x_full[i * SHARD_B : (i + 1) * SHARD_B]} for i in range(N_CORES)
    ]
    res = run_bass_kernel_spmd(nc, in_maps, core_ids=list(range(N_CORES)))
    out = np.concatenate([r["y"] for r in res.results], axis=0)
    return np.ascontiguousarray(out, dtype=np.float32)



# revision 2
# speedup vs baseline: 1.0293x; 1.0293x over previous
"""Trainium2 Bass kernel for a rate-1/2, constraint-length-3 feedforward
convolutional encoder (generator polynomials "101" and "111", MSB-first).

The trellis scan in the reference collapses to elementwise XORs of shifted
input bits (zero initial state):

    out0[t] = u[t] ^ u[t-2]            (poly "101")
    out1[t] = u[t] ^ u[t-1] ^ u[t-2]   (poly "111")

with the codeword interleaved time-major: y[:, 2t] = out0[t], y[:, 2t+1] = out1[t].

V2: SBUF tiles are bf16; the SWDGE (gpsimd) DMA path casts f32<->bf16 in
flight, halving the SBUF-side DMA bytes (0/1 values are exact in bf16).
XOR is computed bitwise on uint16 views (bf16 1.0 = 0x3F80, 0.0 = 0x0000),
one DVE op per output stream. All 8 input slots and 8 output tiles are
SBUF-resident, so every input DMA can be issued upfront with no recycling
dependencies.

Sharding: pure data parallel over the batch dim across 8 NeuronCores.
"""

import numpy as np

N_CORES = 8
B, K = 8192, 2048
N_OUT = 2
SHARD_B = B // N_CORES  # 1024 codewords per core
P = 128                 # SBUF partitions

_compiled = {}


def _build_nc():
    import concourse.bass as bass  # noqa: F401
    import concourse.tile as tile
    from concourse import bacc, mybir

    nc = bacc.Bacc(
        "TRN2",
        target_bir_lowering=False,
        debug=False,
        enable_asserts=False,
    )
    x = nc.dram_tensor("x", [SHARD_B, K], mybir.dt.float32, kind="ExternalInput").ap()
    y = nc.dram_tensor(
        "y", [SHARD_B, N_OUT * K], mybir.dt.float32, kind="ExternalOutput"
    ).ap()

    n_groups = SHARD_B // P  # 8 row-groups of 128

    with tile.TileContext(nc) as tc:
        with (
            tc.tile_pool(name="xin", bufs=1) as in_pool,
            tc.tile_pool(name="out", bufs=1) as out_pool,
        ):
            # Persistent bf16 input slots with 2 leading zero columns so the
            # shifted views u[t-1], u[t-2] fall out of plain column offsets.
            in_slots = [
                in_pool.tile(
                    [P, K + 2], mybir.dt.bfloat16, tag=f"xin{j}", name=f"xin{j}"
                )
                for j in range(n_groups)
            ]
            out_slots = [
                out_pool.tile(
                    [P, N_OUT * K], mybir.dt.bfloat16, tag=f"out{j}", name=f"out{j}"
                )
                for j in range(n_groups)
            ]
            for j in range(n_groups):
                nc.vector.memset(in_slots[j][:, 0:2], 0.0)

            # All input DMAs first: they enqueue on the single SWDGE queue
            # ahead of every output, and the f32->bf16 cast happens in the
            # DMA datapath.
            for g in range(n_groups):
                rows = slice(g * P, (g + 1) * P)
                nc.gpsimd.dma_start(in_slots[g][:, 2 : 2 + K], x[rows, :])

            for g in range(n_groups):
                xin = in_slots[g]
                rows = slice(g * P, (g + 1) * P)
                a = xin[:, 2 : 2 + K].bitcast(mybir.dt.uint16)  # u[t]
                b = xin[:, 1 : 1 + K].bitcast(mybir.dt.uint16)  # u[t-1]
                c = xin[:, 0:K].bitcast(mybir.dt.uint16)        # u[t-2]

                out = out_slots[g]
                even = out[:, 0 : N_OUT * K : 2].bitcast(mybir.dt.uint16)
                odd = out[:, 1 : N_OUT * K : 2].bitcast(mybir.dt.uint16)

                # out0 = a ^ c ; out1 = out0 ^ b  (bitwise on bf16 payloads)
                nc.vector.tensor_tensor(even, a, c, mybir.AluOpType.bitwise_xor)
                nc.vector.tensor_tensor(odd, even, b, mybir.AluOpType.bitwise_xor)

                # bf16 -> f32 cast on the way out.
                nc.gpsimd.dma_start(y[rows, :], out[:])

    nc.compile()
    return nc


def _get_nc():
    if "nc" not in _compiled:
        _compiled["nc"] = _build_nc()
    return _compiled["nc"]


def kernel(**inputs) -> np.ndarray:
    from concourse.bass_utils import run_bass_kernel_spmd

    x_full = np.ascontiguousarray(np.asarray(inputs["inputs"], dtype=np.float32))
    assert x_full.shape == (B, K), x_full.shape

    nc = _get_nc()
    in_maps = [
        {"x": x_full[i * SHARD_B : (i + 1) * SHARD_B]} for i in range(N_CORES)
    ]
    res = run_bass_kernel_spmd(nc, in_maps, core_ids=list(range(N_CORES)))
    out = np.concatenate([r["y"] for r in res.results], axis=0)
    return np.ascontiguousarray(out, dtype=np.float32)


# revision 3
# speedup vs baseline: 1.0339x; 1.0045x over previous
"""Trainium2 Bass kernel for a rate-1/2, constraint-length-3 feedforward
convolutional encoder (generator polynomials "101" and "111", MSB-first).

The trellis scan in the reference collapses to elementwise XORs of shifted
input bits (zero initial state):

    out0[t] = u[t] ^ u[t-2]            (poly "101")
    out1[t] = u[t] ^ u[t-1] ^ u[t-2]   (poly "111")

with the codeword interleaved time-major: y[:, 2t] = out0[t], y[:, 2t+1] = out1[t].

XOR on {0,1} f32 values is computed bitwise on uint32 views (1.0f =
0x3F800000, 0.0f = 0x0), one DVE op per output stream.

DMA layout: the kernel is bound by the ~430 GB/s/core SDMA-engine
aggregate. Inputs are issued upfront, alternating across both HWDGE rings
(SP + ACT) so reads ramp immediately; outputs stream on the SWDGE
(gpsimd) queue and overlap the read phase as soon as the first group's
two XORs finish. All 8 input slots and 8 output tiles are SBUF-resident,
so no DMA ever waits on buffer recycling.

Sharding: pure data parallel over the batch dim across 8 NeuronCores.
"""

import numpy as np

N_CORES = 8
B, K = 8192, 2048
N_OUT = 2
SHARD_B = B // N_CORES  # 1024 codewords per core
P = 128                 # SBUF partitions

_compiled = {}


def _build_nc():
    import concourse.bass as bass  # noqa: F401
    import concourse.tile as tile
    from concourse import bacc, mybir

    nc = bacc.Bacc(
        "TRN2",
        target_bir_lowering=False,
        debug=False,
        enable_asserts=False,
    )
    x = nc.dram_tensor("x", [SHARD_B, K], mybir.dt.float32, kind="ExternalInput").ap()
    y = nc.dram_tensor(
        "y", [SHARD_B, N_OUT * K], mybir.dt.float32, kind="ExternalOutput"
    ).ap()

    n_groups = SHARD_B // P  # 8 row-groups of 128

    with tile.TileContext(nc) as tc:
        with (
            tc.tile_pool(name="xin", bufs=1) as in_pool,
            tc.tile_pool(name="out", bufs=1) as out_pool,
        ):
            # Persistent input slots with 2 leading zero columns so the
            # shifted views u[t-1], u[t-2] fall out of plain column offsets.
            in_slots = [
                in_pool.tile(
                    [P, K + 2], mybir.dt.float32, tag=f"xin{j}", name=f"xin{j}"
                )
                for j in range(n_groups)
            ]
            out_slots = [
                out_pool.tile(
                    [P, N_OUT * K], mybir.dt.float32, tag=f"out{j}", name=f"out{j}"
                )
                for j in range(n_groups)
            ]
            for j in range(n_groups):
                nc.vector.memset(in_slots[j][:, 0:2], 0.0)

            # All input DMAs upfront, alternating between the two HWDGE
            # rings (SP and ACT) so read descriptors stream from two
            # independent queues.
            for g in range(n_groups):
                rows = slice(g * P, (g + 1) * P)
                eng = nc.sync if g % 2 == 0 else nc.scalar
                eng.dma_start(in_slots[g][:, 2 : 2 + K], x[rows, :])

            for g in range(n_groups):
                xin = in_slots[g]
                rows = slice(g * P, (g + 1) * P)
                a = xin[:, 2 : 2 + K].bitcast(mybir.dt.uint32)  # u[t]
                b = xin[:, 1 : 1 + K].bitcast(mybir.dt.uint32)  # u[t-1]
                c = xin[:, 0:K].bitcast(mybir.dt.uint32)        # u[t-2]

                out = out_slots[g]
                even = out[:, 0 : N_OUT * K : 2].bitcast(mybir.dt.uint32)
                odd = out[:, 1 : N_OUT * K : 2].bitcast(mybir.dt.uint32)

                # out0 = a ^ c ; out1 = out0 ^ b  (bitwise on f32 payloads)
                nc.vector.tensor_tensor(even, a, c, mybir.AluOpType.bitwise_xor)
                nc.vector.tensor_tensor(odd, even, b, mybir.AluOpType.bitwise_xor)

                # Output DMAs on the SWDGE path so a stalled input trigger
                # never blocks a ready output DMA (and vice versa).
                nc.gpsimd.dma_start(y[rows, :], out[:])

    nc.compile()
    return nc


def _get_nc():
    if "nc" not in _compiled:
        _compiled["nc"] = _build_nc()
    return _compiled["nc"]


def kernel(**inputs) -> np.ndarray:
    from concourse.bass_utils import run_bass_kernel_spmd

    x_full = np.ascontiguousarray(np.asarray(inputs["inputs"], dtype=np.float32))
    assert x_full.shape == (B, K), x_full.shape

    nc = _get_nc()
    in_maps = [
        {"x": x_full[i * SHARD_B : (i + 1) * SHARD_B]} for i in range(N_CORES)
    ]
    res = run_bass_kernel_spmd(nc, in_maps, core_ids=list(range(N_CORES)))
    out = np.concatenate([r["y"] for r in res.results], axis=0)
    return np.ascontiguousarray(out, dtype=np.float32)
